# revision 16
# baseline (speedup 1.0000x reference)
import sys

sys.path.insert(0, "/opt/trn_rl_repo")

import numpy as np

# Problem constants (nn_Attention_34978213658826)
B, L, DM, NH, DH = 2, 2048, 1024, 16, 64
P = 128
LT = L // P            # 16 q/k tiles
MC = DM // P           # 8 m-chunks
G = 2                  # q-tiles per group for the z matmul
NG = LT // G
HPC = 4                # heads per core
NPAIR = 2              # head pairs per core
NEG = -1.0e30
SCH = 1024             # scores psum chunk width
NQ = L // 512

_CACHE = {}


def _ts(i, n):
    return slice(i * n, (i + 1) * n)


def build_bass():
    import concourse.mybir as mybir
    import concourse.tile as tile
    from concourse import bacc
    from collections import deque

    f32 = mybir.dt.float32
    bf16 = mybir.dt.bfloat16
    AX = mybir.AxisListType
    AF = mybir.ActivationFunctionType

    nc = bacc.Bacc(None, target_bir_lowering=False)
    # x^T split hi/lo in bf16 (hi + lo ~= fp32-accurate contraction)
    xh_d = nc.dram_tensor("xh", [DM, L], bf16, kind="ExternalInput")
    xl_d = nc.dram_tensor("xl", [DM, L], bf16, kind="ExternalInput")
    wq_h = nc.dram_tensor("wqh", [NPAIR, DM + 1, P], bf16, kind="ExternalInput")
    wq_l = nc.dram_tensor("wql", [NPAIR, DM + 1, P], bf16, kind="ExternalInput")
    wk_h = nc.dram_tensor("wkh", [NPAIR, DM + 1, P], bf16, kind="ExternalInput")
    wk_l = nc.dram_tensor("wkl", [NPAIR, DM + 1, P], bf16, kind="ExternalInput")
    wv_d = nc.dram_tensor("wv", [DM + 1, HPC * DH], bf16, kind="ExternalInput")
    wo_d = nc.dram_tensor("wo", [NPAIR, P, DM], bf16, kind="ExternalInput")
    msk = nc.dram_tensor("mask", [P, P], bf16, kind="ExternalInput")
    idn = nc.dram_tensor("ident", [P, P], bf16, kind="ExternalInput")
    out = nc.dram_tensor("out", [L, DM], bf16, kind="ExternalOutput")
    wu_d = nc.dram_tensor("wu", [1, 1], f32, kind="ExternalOutput")

    with tile.TileContext(nc) as tc:
        with (
            tc.tile_pool(name="const", bufs=1) as const,
            tc.tile_pool(name="w", bufs=1) as wp,
            tc.tile_pool(name="xt", bufs=1) as xtp,
            tc.tile_pool(name="qk", bufs=1) as qkp,
            tc.tile_pool(name="prt", bufs=1) as prp,
            tc.tile_pool(name="vz", bufs=1) as vzp,
            tc.tile_pool(name="prow", bufs=3) as prowp,
            tc.tile_pool(name="pt", bufs=2) as ptp,
            tc.tile_pool(name="stat", bufs=8) as statp,
            tc.tile_pool(name="osb", bufs=2) as osbp,
            tc.tile_pool(name="pp_ps", bufs=2, space="PSUM") as pp_ps,
            tc.tile_pool(name="s_ps", bufs=2, space="PSUM") as s_ps,
            tc.tile_pool(name="zo_ps", bufs=2, space="PSUM") as zo_ps,
        ):
            ident = const.tile([P, P], bf16)
            mask = const.tile([P, P], bf16)
            garb = const.tile([P, P], bf16)
            ones = const.tile([2, 512], bf16)
            nc.vector.memset(garb, 0.5)
            nc.vector.memset(ones, 1.0)

            # weights: [partition=m-row, pair, m-chunk, headcol]
            wqk = {}
            for nm in ("qh", "ql", "kh", "kl"):
                wqk[nm] = wp.tile([P, NPAIR, MC, P], bf16, name=f"w{nm}", tag=f"w{nm}")
            bq2 = wp.tile([2, NPAIR, P], bf16)
            bk2 = wp.tile([2, NPAIR, P], bf16)
            wv_t = wp.tile([P, MC, HPC * DH], bf16)
            wv_b = wp.tile([1, HPC * DH], bf16)
            wo_t = wp.tile([P, NPAIR, DM], bf16)

            xh = xtp.tile([P, MC, L], bf16)
            xl = xtp.tile([P, MC, L], bf16)

            vv = vzp.tile([P, LT, HPC * DH], bf16)
            zst = [vzp.tile([P, NPAIR, G * P], bf16, name=f"zst{g}", tag=f"zst{g}") for g in range(NG)]

            # ---------------- DMA emission (sync queue paces x chunks) -----
            nc.sync.dma_start(xh[:, 0], xh_d[_ts(0, P), :])
            nc.sync.dma_start(xl[:, 0], xl_d[_ts(0, P), :])
            for nm, dram in (("qh", wq_h), ("ql", wq_l), ("kh", wk_h), ("kl", wk_l)):
                nc.sync.dma_start(
                    wqk[nm][:, 0], dram[0, :DM, :].rearrange("(c p) h -> p c h", p=P)
                )
            nc.sync.dma_start(bq2[0:1, 0], wq_h[0, DM : DM + 1, :])
            nc.sync.dma_start(bq2[1:2, 0], wq_l[0, DM : DM + 1, :])
            nc.sync.dma_start(bk2[0:1, 0], wk_h[0, DM : DM + 1, :])
            nc.sync.dma_start(bk2[1:2, 0], wk_l[0, DM : DM + 1, :])
            for m in range(1, MC):
                nc.sync.dma_start(xh[:, m], xh_d[_ts(m, P), :])
                nc.sync.dma_start(xl[:, m], xl_d[_ts(m, P), :])
            nc.sync.dma_start(ident, idn[:, :])
            nc.sync.dma_start(mask, msk[:, :])
            for nm, dram in (("qh", wq_h), ("ql", wq_l), ("kh", wk_h), ("kl", wk_l)):
                nc.sync.dma_start(
                    wqk[nm][:, 1], dram[1, :DM, :].rearrange("(c p) h -> p c h", p=P)
                )
            nc.sync.dma_start(bq2[0:1, 1], wq_h[1, DM : DM + 1, :])
            nc.sync.dma_start(bq2[1:2, 1], wq_l[1, DM : DM + 1, :])
            nc.sync.dma_start(bk2[0:1, 1], wk_h[1, DM : DM + 1, :])
            nc.sync.dma_start(bk2[1:2, 1], wk_l[1, DM : DM + 1, :])
            nc.sync.dma_start(wv_t, wv_d[:DM, :].rearrange("(c p) h -> p c h", p=P))
            nc.sync.dma_start(wv_b, wv_d[DM : DM + 1, :])
            for _pr in range(NPAIR):
                nc.sync.dma_start(wo_t[:, _pr], wo_d[_pr, :, :])

            # -------- PE warm-up on memset data (no DMA dependency) --------
            wup_st = tc.tile_pool(name="wupp", bufs=1)
            wupp = wup_st.__enter__()
            wup = wupp.tile([1, 4], f32)
            wps = pp_ps.tile([P, 512], f32, name="wps", tag="pp")
            for w_ in range(104):
                nc.tensor.matmul(
                    wps[:, :P], lhsT=garb, rhs=garb,
                    start=(w_ == 0), stop=(w_ == 103),
                )
            nc.vector.reduce_max(wup[:1, :1], wps[:1, :P], axis=AX.X)
            nc.sync.dma_start(wu_d[:, :], wup[:1, :1])
            wup_st.__exit__(None, None, None)

            # per-pair rotating tiles (pool reuses buffers across pairs)
            def alloc_qT():
                return {
                    "qh": prp.tile([P, L], bf16, name="qTh", tag="qTh"),
                    "ql": prp.tile([P, L], bf16, name="qTl", tag="qTl"),
                    "kh": prp.tile([P, L], bf16, name="kTh", tag="kTh"),
                    "kl": prp.tile([P, L], bf16, name="kTl", tag="kTl"),
                }

            def alloc_qkS():
                return {
                    "qS": qkp.tile([P, 2, L], bf16, name="qS", tag="qS"),
                    "kS": qkp.tile([P, 2, L], bf16, name="kS", tag="kS"),
                    "kSw": qkp.tile([P, 2, L], bf16, name="kSw", tag="kSw"),
                }

            qT = {}
            qkS = {}

            def proj_evac(pr, w_, n, ps):
                # bias matmul + psum -> hi/lo sbuf rows for chain (w_, n)
                b2 = bq2 if w_ == "q" else bk2
                nc.tensor.matmul(
                    ps, lhsT=b2[:, pr, :], rhs=ones[:, :512],
                    start=False, stop=True,
                )
                dh = qT[pr]["qh" if w_ == "q" else "kh"]
                dl = qT[pr]["ql" if w_ == "q" else "kl"]
                nc.scalar.copy(dh[:, _ts(n, 512)], ps)
                nc.vector.scalar_tensor_tensor(
                    dl[:, _ts(n, 512)], ps, 1.0, dh[:, _ts(n, 512)],
                    op0=mybir.AluOpType.mult,
                    op1=mybir.AluOpType.subtract,
                )

            def emit_builds(pr):
                # per-head stacked tiles via sbuf-sbuf DMA
                t = qkS[pr]
                for h2 in range(2):
                    hs = _ts(h2, DH)
                    nc.sync.dma_start(t["qS"][0:DH, h2, :], qT[pr]["qh"][hs, :])
                    nc.sync.dma_start(t["qS"][DH:P, h2, :], qT[pr]["ql"][hs, :])
                    nc.sync.dma_start(t["kS"][0:DH, h2, :], qT[pr]["kh"][hs, :])
                    nc.sync.dma_start(t["kS"][DH:P, h2, :], qT[pr]["kl"][hs, :])
                    nc.sync.dma_start(t["kSw"][0:DH, h2, :], qT[pr]["kl"][hs, :])
                    nc.sync.dma_start(t["kSw"][DH:P, h2, :], qT[pr]["kh"][hs, :])

            # ------------- stage 0: pair-0 q/k projection, 8 banks ---------
            qT[0] = alloc_qT()
            ps_s = [s_ps.tile([P, SCH], f32, name="s0p", tag="s") for _ in range(2)]
            ps_p = [pp_ps.tile([P, 512], f32, name="p0p", tag="pp") for _ in range(2)]
            ps_z = [zo_ps.tile([P, 512], f32, name="z0p", tag="zo") for _ in range(2)]
            chain_ps = {
                ("q", 0): ps_p[0], ("q", 1): ps_p[1],
                ("q", 2): ps_s[0][:, 0:512], ("q", 3): ps_s[0][:, 512:1024],
                ("k", 0): ps_s[1][:, 0:512], ("k", 1): ps_s[1][:, 512:1024],
                ("k", 2): ps_z[0], ("k", 3): ps_z[1],
            }
            for m in range(MC):
                for w_ in ("q", "k"):
                    th = wqk[w_ + "h"]
                    tl = wqk[w_ + "l"]
                    for vi, (lw, rx) in enumerate((
                        (th[:, 0, m, :], xh),
                        (tl[:, 0, m, :], xh),
                        (th[:, 0, m, :], xl),
                    )):
                        for n in range(NQ):
                            nc.tensor.matmul(
                                chain_ps[(w_, n)], lhsT=lw,
                                rhs=rx[:, m, _ts(n, 512)],
                                start=(m == 0 and vi == 0), stop=False,
                            )
            for w_ in ("q", "k"):
                for n in range(NQ):
                    proj_evac(0, w_, n, chain_ps[(w_, n)])
            qkS[0] = alloc_qkS()
            emit_builds(0)
            # pair-1 projection targets (pool rotation: reuses pair-0 buffers
            # once the pair-0 builds above have consumed them)
            qT[1] = alloc_qT()

            # ------------- stage-1 filler generators -----------------------
            # v-projection tiles (PE filler; evac on ACT)
            v_emitted = [0]

            def emit_v_tile(lt):
                ps = pp_ps.tile([P, 512], f32, name="vps", tag="pp")
                for m in range(MC):
                    nc.tensor.matmul(
                        ps[:, : HPC * DH],
                        lhsT=xh[:, m, _ts(lt, P)], rhs=wv_t[:, m, :],
                        start=(m == 0), stop=False,
                    )
                nc.tensor.matmul(
                    ps[:, : HPC * DH],
                    lhsT=ones[0:1, :P], rhs=wv_b,
                    start=False, stop=True,
                )
                nc.scalar.copy(vv[:, lt, :], ps[:, : HPC * DH])
                v_emitted[0] += 1

            # pair-1 projection chains, emitted as half-chain pieces
            p1_chains = deque([(w, n) for w in ("q", "k") for n in range(NQ)])
            p1_open = [None, None, 0]  # (w, n), psum, m-pos

            def emit_p1_piece():
                if p1_open[0] is None:
                    if not p1_chains:
                        return False
                    p1_open[0] = p1_chains.popleft()
                    p1_open[1] = pp_ps.tile([P, 512], f32, name="p1p", tag="pp")
                    p1_open[2] = 0
                w_, n = p1_open[0]
                ps = p1_open[1]
                m0 = p1_open[2]
                th = wqk[w_ + "h"]
                tl = wqk[w_ + "l"]
                for m in range(m0, m0 + 4):
                    for vi, (lw, rx) in enumerate((
                        (th[:, 1, m, :], xh),
                        (tl[:, 1, m, :], xh),
                        (th[:, 1, m, :], xl),
                    )):
                        nc.tensor.matmul(
                            ps, lhsT=lw, rhs=rx[:, m, _ts(n, 512)],
                            start=(m == 0 and vi == 0), stop=False,
                        )
                p1_open[2] = m0 + 4
                if p1_open[2] >= MC:
                    proj_evac(1, w_, n, ps)
                    p1_open[0] = None
                return True

            def p1_left():
                return p1_chains or p1_open[0] is not None

            # ---------------- fused score/z/out stages ----------------
            ptgs = {}

            def emit_S_front(pr, g, s, h2):
                # one (head, q-tile): score matmuls + per-chunk max + exp
                if s == 0 and h2 == 0:
                    # layout [P, s, j, P]: transpose writes are contiguous
                    # (i+1)*256B runs per partition instead of 256B
                    ptgs[(pr, g)] = [
                        ptp.tile([P, G, LT, P], bf16, name=f"ptg{j}", tag=f"ptg{j}")
                        for j in range(2)
                    ]
                t = qkS[pr]
                i = g * G + s
                klen = (i + 1) * P
                nch = (klen + SCH - 1) // SCH
                lq = t["qS"][:, h2, _ts(i, P)]
                prow = prowp.tile([P, L], bf16)
                negmc = statp.tile([P, 4], f32, tag="negmc")
                sums = statp.tile([P, 4], f32, tag="sums")
                for c in range(nch):
                    cw = min(SCH, klen - c * SCH)
                    dlo = klen - P - c * SCH  # diag block offset in chunk
                    sp = s_ps.tile([P, SCH], f32, name="sp", tag="s")
                    for w0 in range(0, cw, 512):
                        ww = min(512, cw - w0)
                        has_diag = w0 <= dlo < w0 + ww
                        nc.tensor.matmul(
                            sp[:, w0 : w0 + ww], lhsT=lq,
                            rhs=t["kS"][:, h2, c * SCH + w0 : c * SCH + w0 + ww],
                            start=True, stop=False,
                        )
                        nc.tensor.matmul(
                            sp[:, w0 : w0 + ww], lhsT=lq,
                            rhs=t["kSw"][:, h2, c * SCH + w0 : c * SCH + w0 + ww],
                            start=False, stop=not has_diag,
                        )
                        if has_diag:
                            nc.tensor.matmul(
                                sp[:, dlo : dlo + P], lhsT=ident, rhs=mask,
                                start=False, stop=True,
                            )
                    nc.vector.reduce_max(
                        negmc[:, c : c + 1], sp[:, :cw], axis=AX.X, negate=True
                    )
                    nc.scalar.activation(
                        prow[:, c * SCH : c * SCH + cw],
                        sp[:, :cw],
                        AF.Exp,
                        bias=negmc[:, c : c + 1],
                        accum_out=sums[:, c : c + 1],
                    )
                return (pr, g, s, h2, prow, negmc, sums, nch, klen)

            def emit_S_back(ctx):
                # deferred per-unit tail: global rescale + 1/sum + transpose
                pr, g, s, h2, prow, negmc, sums, nch, klen = ctx
                i = g * G + s
                ptg = ptgs[(pr, g)]
                sinv = statp.tile([P, 1], f32, tag="sinv")
                if nch > 1:
                    negmg = statp.tile([P, 1], f32, tag="negmg")
                    nc.vector.tensor_reduce(
                        negmg, negmc[:, :nch], axis=AX.X, op=mybir.AluOpType.min
                    )
                    rsc = statp.tile([P, 4], f32, tag="rsc")
                    nc.scalar.activation(
                        rsc[:, :nch], negmc[:, :nch], AF.Exp,
                        bias=negmg, scale=-1.0,
                    )
                    ssc = statp.tile([P, 4], f32, tag="ssc")
                    nc.vector.tensor_mul(ssc[:, :nch], sums[:, :nch], rsc[:, :nch])
                    stot = statp.tile([P, 1], f32, tag="stot")
                    nc.vector.reduce_sum(stot, ssc[:, :nch], axis=AX.X)
                    nc.vector.reciprocal(sinv, stot)
                    wsc = statp.tile([P, 4], f32, tag="wsc")
                    nc.vector.tensor_scalar_mul(wsc[:, :nch], rsc[:, :nch], sinv)
                    for c in range(nch):
                        cw = min(SCH, klen - c * SCH)
                        nc.vector.tensor_scalar_mul(
                            prow[:, c * SCH : c * SCH + cw],
                            prow[:, c * SCH : c * SCH + cw],
                            wsc[:, c : c + 1],
                        )
                else:
                    nc.vector.reciprocal(sinv, sums[:, :1])
                    nc.vector.tensor_scalar_mul(
                        prow[:, :klen], prow[:, :klen], sinv
                    )
                nc.sync.dma_start_transpose(
                    ptg[h2][:, s, : i + 1, :], prow[:, :klen]
                )

            def emit_Z_h(pr, g, h2):
                ptg = ptgs[(pr, g)]
                hcol = (pr * 2 + h2) * DH
                zps = zo_ps.tile([DH, G * P], f32, name="zps", tag="zo")
                jmax = G * (g + 1)
                for j in range(jmax):
                    sc = max(0, j - G * g)
                    nc.tensor.matmul(
                        zps[:, sc * P :],
                        lhsT=vv[:, j, hcol : hcol + DH],
                        rhs=ptg[h2][:, sc:G, j, :],
                        start=(j == 0),
                        stop=(j == jmax - 1),
                    )
                nc.vector.tensor_scalar_mul(zst[g][_ts(h2, DH), pr, :], zps, 1.0)

            def emit_O_qtile(g, s):
                i = g * G + s
                osb = osbp.tile([P, DM], bf16)
                for mc2 in range(2):
                    ops = zo_ps.tile([P, 512], f32, name="ops", tag="zo")
                    for pr in range(NPAIR):
                        nc.tensor.matmul(
                            ops,
                            lhsT=zst[g][:, pr, _ts(s, P)],
                            rhs=wo_t[:, pr, _ts(mc2, 512)],
                            start=(pr == 0),
                            stop=(pr == 1),
                        )
                    if mc2 == 0:
                        nc.scalar.copy(osb[:, _ts(mc2, 512)], ops)
                    else:
                        nc.vector.tensor_scalar_mul(osb[:, _ts(mc2, 512)], ops, 1.0)
                nc.gpsimd.dma_start(out[_ts(i, P), :], osb)

            # Deferred z / out-proj work popped between score units so the
            # PE always has independent matmuls while softmax drains.
            zq = deque()
            oq = deque()
            epoch = [0]

            def z_poppable(drain):
                # z(pr,g) may only be emitted once its group's transposes are
                # queued (epoch gate) and its vv tiles are already emitted
                # (PE-queue FIFO would deadlock otherwise).
                if not zq:
                    return False
                ep, (pr_, g_, h2_) = zq[0]
                if not (drain or ep <= epoch[0] - 1):
                    return False
                return v_emitted[0] >= G * (g_ + 1)

            def pump(budget, stage, drain=False):
                while budget > 0:
                    chain_open = p1_open[0] is not None
                    need_v = min(G * (epoch[0] + 2), LT)
                    if stage == 1 and v_emitted[0] < need_v and not chain_open:
                        # v tiles needed by upcoming z chains; a new pp-pool
                        # alloc is only safe with no half-done p1 chain open
                        emit_v_tile(v_emitted[0])
                        budget -= 1
                    elif z_poppable(drain):
                        pr_, g_, h2_ = zq.popleft()[1]
                        emit_Z_h(pr_, g_, h2_)
                        if pr_ == 1 and h2_ == 1:
                            for s_ in range(G):
                                oq.append((g_, s_))
                        budget -= max(1, (G * (g_ + 1)) // 6)
                    elif stage == 1 and p1_left():
                        emit_p1_piece()
                        budget -= 2
                    elif stage == 1 and v_emitted[0] < LT and not chain_open:
                        emit_v_tile(v_emitted[0])
                        budget -= 1
                    elif stage == 2 and oq:
                        emit_O_qtile(*oq.popleft())
                        budget -= 1
                    else:
                        return

            pending = [None]

            def emit_S(pr, g, stage):
                # safety drain: a z chain two epochs stale must be emitted
                # before this block's ptg alloc (PE-queue FIFO would
                # deadlock on the ptg pool rotation otherwise)
                while zq and zq[0][0] <= epoch[0] - 2:
                    _, (pr_, g_, h2_) = zq[0]
                    while p1_open[0] is not None:
                        emit_p1_piece()
                    while v_emitted[0] < G * (g_ + 1):
                        emit_v_tile(v_emitted[0])
                    zq.popleft()
                    emit_Z_h(pr_, g_, h2_)
                    if pr_ == 1 and h2_ == 1:
                        for s_ in range(G):
                            oq.append((g_, s_))
                # software pipeline: unit u+1's matmuls+max+exp are emitted
                # before unit u's stat/rescale/transpose tail.
                for s_ in range(G):
                    for h2 in range(2):
                        ctx = emit_S_front(pr, g, s_, h2)
                        if pending[0] is not None:
                            emit_S_back(pending[0])
                        pending[0] = ctx
                        pump(3, stage)
                for h2 in range(2):
                    zq.append((epoch[0], (pr, g, h2)))
                epoch[0] += 1

            # stage 1: pair-0 units with proj-1/v/z filler
            for g in range(NG):
                emit_S(0, g, stage=1)

            # boundary: flush pipeline tail, drain remaining stage-1 filler
            if pending[0] is not None:
                emit_S_back(pending[0])
                pending[0] = None
            while p1_left():
                emit_p1_piece()
            while v_emitted[0] < LT:
                emit_v_tile(v_emitted[0])
            qkS[1] = alloc_qkS()
            emit_builds(1)
            # z(0,3,*) drains here as boundary filler, covering the builds
            while zq:
                pr_, g_, h2_ = zq.popleft()[1]
                emit_Z_h(pr_, g_, h2_)

            # stage 2: pair-1 units with z/out filler
            for g in range(NG):
                emit_S(1, g, stage=2)
            if pending[0] is not None:
                emit_S_back(pending[0])
                pending[0] = None
            while zq:
                pump(1, stage=2, drain=True)
                for _ in range(4):
                    if oq:
                        emit_O_qtile(*oq.popleft())
            while oq:
                emit_O_qtile(*oq.popleft())

    nc.finalize()
    return nc


def _split_bf16(a):
    import ml_dtypes

    hi = a.astype(ml_dtypes.bfloat16)
    lo = (a - hi.astype(np.float32)).astype(ml_dtypes.bfloat16)
    return hi, lo


def make_in_maps(normal_pre_resid, W_Q, W_K, W_V, W_O, b_Q, b_K, b_V, b_O):
    import ml_dtypes

    x = np.asarray(normal_pre_resid, np.float32)
    W_Q = np.asarray(W_Q, np.float32) * 0.125  # fold 1/sqrt(d_head)
    W_K = np.asarray(W_K, np.float32)
    W_V = np.asarray(W_V, np.float32)
    W_O = np.asarray(W_O, np.float32)
    b_Q = np.asarray(b_Q, np.float32) * 0.125
    b_K = np.asarray(b_K, np.float32)
    b_V = np.asarray(b_V, np.float32)

    mask = np.triu(np.full((P, P), NEG, np.float32), k=1).astype(ml_dtypes.bfloat16)
    ident = np.eye(P, dtype=np.float32).astype(ml_dtypes.bfloat16)
    in_maps = []
    for c in range(8):
        b, hg = divmod(c, 4)
        heads = [4 * hg + j for j in range(HPC)]
        xT = np.ascontiguousarray(x[b].T)  # [DM, L]
        xh, xl = _split_bf16(xT)

        def pack_qk(W, bias):
            prs = []
            for p_ in range(NPAIR):
                h0, h1 = heads[2 * p_], heads[2 * p_ + 1]
                wcat = np.concatenate([W[h0], W[h1]], axis=1)  # [DM, 128]
                bcat = np.concatenate([bias[h0], bias[h1]])[None, :]
                prs.append(np.concatenate([wcat, bcat], axis=0))  # [DM+1, 128]
            return _split_bf16(np.ascontiguousarray(np.stack(prs)))

        wqh, wql = pack_qk(W_Q, b_Q)
        wkh, wkl = pack_qk(W_K, b_K)
        wv_cat = np.concatenate([W_V[h] for h in heads], axis=1)
        bv_cat = np.concatenate([b_V[h] for h in heads])[None, :]
        wv_full = np.concatenate([wv_cat, bv_cat], axis=0).astype(ml_dtypes.bfloat16)
        wo_prs = np.ascontiguousarray(
            np.stack(
                [
                    np.concatenate(
                        [W_O[heads[2 * p_]], W_O[heads[2 * p_ + 1]]], axis=0
                    )
                    for p_ in range(NPAIR)
                ]
            )
        ).astype(ml_dtypes.bfloat16)  # [2, 128, DM]

        in_maps.append(
            {
                "xh": np.ascontiguousarray(xh),
                "xl": np.ascontiguousarray(xl),
                "wqh": wqh,
                "wql": wql,
                "wkh": wkh,
                "wkl": wkl,
                "wv": np.ascontiguousarray(wv_full),
                "wo": wo_prs,
                "mask": mask,
                "ident": ident,
            }
        )
    return in_maps


def run_device(in_maps, **kwargs):
    from concourse.bass_utils import run_bass_kernel_spmd

    if "nc" not in _CACHE:
        _CACHE["nc"] = build_bass()
    return run_bass_kernel_spmd(_CACHE["nc"], in_maps, core_ids=list(range(8)), **kwargs)


def kernel(normal_pre_resid, W_Q, W_K, W_V, W_O, b_Q, b_K, b_V, b_O, **extra):
    b_O = np.asarray(b_O, np.float32)
    in_maps = make_in_maps(
        normal_pre_resid, W_Q, W_K, W_V, W_O, b_Q, b_K, b_V, b_O
    )
    res = run_device(in_maps)
    outs = [r["out"] for r in res.results]
    full = np.zeros((B, L, DM), np.float32)
    for c in range(8):
        full[c // 4] += outs[c].astype(np.float32)
    full += b_O[None, None, :]
    return full


# revision 21
# speedup vs baseline: 1.0710x; 1.0710x over previous
import sys

sys.path.insert(0, "/opt/trn_rl_repo")

import numpy as np

# Problem constants (nn_Attention_34978213658826)
B, L, DM, NH, DH = 2, 2048, 1024, 16, 64
P = 128
LT = L // P            # 16 q/k tiles
MC = DM // P           # 8 m-chunks
G = 2                  # q-tiles per group for the z matmul
NG = LT // G
HPC = 4                # heads per core
NPAIR = 2              # head pairs per core
NEG = -1.0e30
SCH = 1024             # scores psum chunk width
NQ = L // 512

_CACHE = {}


def _ts(i, n):
    return slice(i * n, (i + 1) * n)


def build_bass():
    import concourse.mybir as mybir
    import concourse.tile as tile
    from concourse import bacc
    from collections import deque

    f32 = mybir.dt.float32
    bf16 = mybir.dt.bfloat16
    AX = mybir.AxisListType
    AF = mybir.ActivationFunctionType

    nc = bacc.Bacc(None, target_bir_lowering=False)
    # x^T split hi/lo in bf16 (hi + lo ~= fp32-accurate contraction)
    xh_d = nc.dram_tensor("xh", [DM, L], bf16, kind="ExternalInput")
    xl_d = nc.dram_tensor("xl", [DM, L], bf16, kind="ExternalInput")
    wq_h = nc.dram_tensor("wqh", [NPAIR, DM + 1, P], bf16, kind="ExternalInput")
    wq_l = nc.dram_tensor("wql", [NPAIR, DM + 1, P], bf16, kind="ExternalInput")
    wk_h = nc.dram_tensor("wkh", [NPAIR, DM + 1, P], bf16, kind="ExternalInput")
    wk_l = nc.dram_tensor("wkl", [NPAIR, DM + 1, P], bf16, kind="ExternalInput")
    wv_d = nc.dram_tensor("wv", [DM + 1, HPC * DH], bf16, kind="ExternalInput")
    wo_d = nc.dram_tensor("wo", [NPAIR, P, DM], bf16, kind="ExternalInput")
    msk = nc.dram_tensor("mask", [P, P], bf16, kind="ExternalInput")
    idn = nc.dram_tensor("ident", [P, P], bf16, kind="ExternalInput")
    out = nc.dram_tensor("out", [L, DM], bf16, kind="ExternalOutput")
    wu_d = nc.dram_tensor("wu", [1, 1], f32, kind="ExternalOutput")

    with tile.TileContext(nc) as tc:
        with (
            tc.tile_pool(name="const", bufs=1) as const,
            tc.tile_pool(name="w", bufs=1) as wp,
            tc.tile_pool(name="xt", bufs=1) as xtp,
            tc.tile_pool(name="qk", bufs=1) as qkp,
            tc.tile_pool(name="stg", bufs=2) as stgp,
            tc.tile_pool(name="vz", bufs=1) as vzp,
            tc.tile_pool(name="prow", bufs=3) as prowp,
            tc.tile_pool(name="pt", bufs=2) as ptp,
            tc.tile_pool(name="stat", bufs=8) as statp,
            tc.tile_pool(name="osb", bufs=2) as osbp,
            tc.tile_pool(name="pp_ps", bufs=2, space="PSUM") as pp_ps,
            tc.tile_pool(name="s_ps", bufs=2, space="PSUM") as s_ps,
            tc.tile_pool(name="zo_ps", bufs=2, space="PSUM") as zo_ps,
        ):
            ident = const.tile([P, P], bf16)
            mask = const.tile([P, P], bf16)
            garb = const.tile([P, P], bf16)
            ones = const.tile([2, 512], bf16)
            nc.vector.memset(garb, 0.5)
            nc.vector.memset(ones, 1.0)

            # weights: [partition=m-row, pair, m-chunk, headcol]
            wqk = {}
            for nm in ("qh", "ql", "kh", "kl"):
                wqk[nm] = wp.tile([P, NPAIR, MC, P], bf16, name=f"w{nm}", tag=f"w{nm}")
            bq2 = wp.tile([2, NPAIR, P], bf16)
            bk2 = wp.tile([2, NPAIR, P], bf16)
            wv_t = wp.tile([P, MC, HPC * DH], bf16)
            wv_b = wp.tile([1, HPC * DH], bf16)
            wo_t = wp.tile([P, NPAIR, DM], bf16)

            xh = xtp.tile([P, MC, L], bf16)
            xl = xtp.tile([P, MC, L], bf16)

            vv = vzp.tile([P, LT, HPC * DH], bf16)
            zst = [vzp.tile([P, NPAIR, G * P], bf16, name=f"zst{g}", tag=f"zst{g}") for g in range(NG)]

            # ---------------- DMA emission (sync queue paces x chunks) -----
            nc.sync.dma_start(xh[:, 0], xh_d[_ts(0, P), :])
            nc.sync.dma_start(xl[:, 0], xl_d[_ts(0, P), :])
            for nm, dram in (("qh", wq_h), ("ql", wq_l), ("kh", wk_h), ("kl", wk_l)):
                nc.sync.dma_start(
                    wqk[nm][:, 0], dram[0, :DM, :].rearrange("(c p) h -> p c h", p=P)
                )
            nc.sync.dma_start(bq2[0:1, 0], wq_h[0, DM : DM + 1, :])
            nc.sync.dma_start(bq2[1:2, 0], wq_l[0, DM : DM + 1, :])
            nc.sync.dma_start(bk2[0:1, 0], wk_h[0, DM : DM + 1, :])
            nc.sync.dma_start(bk2[1:2, 0], wk_l[0, DM : DM + 1, :])
            for m in range(1, MC):
                nc.sync.dma_start(xh[:, m], xh_d[_ts(m, P), :])
                nc.sync.dma_start(xl[:, m], xl_d[_ts(m, P), :])
            nc.sync.dma_start(ident, idn[:, :])
            nc.sync.dma_start(mask, msk[:, :])
            for nm, dram in (("qh", wq_h), ("ql", wq_l), ("kh", wk_h), ("kl", wk_l)):
                nc.sync.dma_start(
                    wqk[nm][:, 1], dram[1, :DM, :].rearrange("(c p) h -> p c h", p=P)
                )
            nc.sync.dma_start(bq2[0:1, 1], wq_h[1, DM : DM + 1, :])
            nc.sync.dma_start(bq2[1:2, 1], wq_l[1, DM : DM + 1, :])
            nc.sync.dma_start(bk2[0:1, 1], wk_h[1, DM : DM + 1, :])
            nc.sync.dma_start(bk2[1:2, 1], wk_l[1, DM : DM + 1, :])
            nc.sync.dma_start(wv_t, wv_d[:DM, :].rearrange("(c p) h -> p c h", p=P))
            nc.sync.dma_start(wv_b, wv_d[DM : DM + 1, :])
            for _pr in range(NPAIR):
                nc.sync.dma_start(wo_t[:, _pr], wo_d[_pr, :, :])

            # -------- PE warm-up on memset data (no DMA dependency) --------
            wup_st = tc.tile_pool(name="wupp", bufs=1)
            wupp = wup_st.__enter__()
            wup = wupp.tile([1, 4], f32)
            wps = pp_ps.tile([P, 512], f32, name="wps", tag="pp")
            for w_ in range(104):
                nc.tensor.matmul(
                    wps[:, :P], lhsT=garb, rhs=garb,
                    start=(w_ == 0), stop=(w_ == 103),
                )
            nc.vector.reduce_max(wup[:1, :1], wps[:1, :P], axis=AX.X)
            nc.sync.dma_start(wu_d[:, :], wup[:1, :1])
            wup_st.__exit__(None, None, None)

            # both pairs' stacked tiles resident: [rows 0:64 | 64:128] per head
            # qS = [qh; ql], kS = [kh; kl], kSw = [kl; kh]
            qkS = {
                pr: {
                    "qS": qkp.tile([P, 2, L], bf16, name=f"qS{pr}", tag=f"qS{pr}"),
                    "kS": qkp.tile([P, 2, L], bf16, name=f"kS{pr}", tag=f"kS{pr}"),
                    "kSw": qkp.tile([P, 2, L], bf16, name=f"kSw{pr}", tag=f"kSw{pr}"),
                }
                for pr in range(NPAIR)
            }
            LO = slice(DH, P)
            HI = slice(0, DH)

            def proj_evac(pr, w_, n, ps):
                # bias matmul, then evacuate psum [h0|h1 cols] directly into
                # the stacked layouts: same-partition halves via ACT/DVE,
                # cross-partition halves via small sbuf-sbuf DMAs.
                b2 = bq2 if w_ == "q" else bk2
                nc.tensor.matmul(
                    ps, lhsT=b2[:, pr, :], rhs=ones[:, :512],
                    start=False, stop=True,
                )
                nsl = _ts(n, 512)
                if w_ == "q":
                    qS = qkS[pr]["qS"]
                    stg = stgp.tile([P, 512], bf16, name="stg", tag="stg")
                    # hi: h0 direct, h1 staged then shifted down->up
                    nc.scalar.copy(qS[HI, 0, nsl], ps[HI, :])
                    nc.scalar.copy(stg[LO, :], ps[LO, :])
                    nc.sync.dma_start(qS[HI, 1, nsl], stg[LO, :])
                    # lo = ps - hi: h1 direct, h0 staged
                    nc.vector.scalar_tensor_tensor(
                        qS[LO, 1, nsl], ps[LO, :], 1.0, stg[LO, :],
                        op0=mybir.AluOpType.mult, op1=mybir.AluOpType.subtract,
                    )
                    nc.vector.scalar_tensor_tensor(
                        stg[HI, :], ps[HI, :], 1.0, qS[HI, 0, nsl],
                        op0=mybir.AluOpType.mult, op1=mybir.AluOpType.subtract,
                    )
                    nc.sync.dma_start(qS[LO, 0, nsl], stg[HI, :])
                else:
                    kS = qkS[pr]["kS"]
                    kSw = qkS[pr]["kSw"]
                    # hi: kh(h0) -> kS[HI,0] direct; kh(h1) -> kSw[LO,1] direct
                    nc.scalar.copy(kS[HI, 0, nsl], ps[HI, :])
                    nc.scalar.copy(kSw[LO, 1, nsl], ps[LO, :])
                    # lo: kl(h0) -> kSw[HI,0] direct; kl(h1) -> kS[LO,1] direct
                    nc.vector.scalar_tensor_tensor(
                        kSw[HI, 0, nsl], ps[HI, :], 1.0, kS[HI, 0, nsl],
                        op0=mybir.AluOpType.mult, op1=mybir.AluOpType.subtract,
                    )
                    nc.vector.scalar_tensor_tensor(
                        kS[LO, 1, nsl], ps[LO, :], 1.0, kSw[LO, 1, nsl],
                        op0=mybir.AluOpType.mult, op1=mybir.AluOpType.subtract,
                    )
                    # cross halves are copies of already-written halves
                    nc.sync.dma_start(kSw[LO, 0, nsl], kS[HI, 0, nsl])
                    nc.sync.dma_start(kS[HI, 1, nsl], kSw[LO, 1, nsl])
                    nc.sync.dma_start(kS[LO, 0, nsl], kSw[HI, 0, nsl])
                    nc.sync.dma_start(kSw[HI, 1, nsl], kS[LO, 1, nsl])

            # ------------- stage 0: pair-0 q/k projection, 8 banks ---------
            ps_s = [s_ps.tile([P, SCH], f32, name="s0p", tag="s") for _ in range(2)]
            ps_p = [pp_ps.tile([P, 512], f32, name="p0p", tag="pp") for _ in range(2)]
            ps_z = [zo_ps.tile([P, 512], f32, name="z0p", tag="zo") for _ in range(2)]
            chain_ps = {
                ("q", 0): ps_p[0], ("q", 1): ps_p[1],
                ("q", 2): ps_s[0][:, 0:512], ("q", 3): ps_s[0][:, 512:1024],
                ("k", 0): ps_s[1][:, 0:512], ("k", 1): ps_s[1][:, 512:1024],
                ("k", 2): ps_z[0], ("k", 3): ps_z[1],
            }
            for m in range(MC):
                for w_ in ("q", "k"):
                    th = wqk[w_ + "h"]
                    tl = wqk[w_ + "l"]
                    for vi, (lw, rx) in enumerate((
                        (th[:, 0, m, :], xh),
                        (tl[:, 0, m, :], xh),
                        (th[:, 0, m, :], xl),
                    )):
                        for n in range(NQ):
                            nc.tensor.matmul(
                                chain_ps[(w_, n)], lhsT=lw,
                                rhs=rx[:, m, _ts(n, 512)],
                                start=(m == 0 and vi == 0), stop=False,
                            )
            for w_ in ("q", "k"):
                for n in range(NQ):
                    proj_evac(0, w_, n, chain_ps[(w_, n)])

            # ------------- stage-1 filler generators -----------------------
            # v-projection tiles (PE filler; evac on ACT)
            v_emitted = [0]

            def emit_v_tile(lt):
                ps = pp_ps.tile([P, 512], f32, name="vps", tag="pp")
                for m in range(MC):
                    nc.tensor.matmul(
                        ps[:, : HPC * DH],
                        lhsT=xh[:, m, _ts(lt, P)], rhs=wv_t[:, m, :],
                        start=(m == 0), stop=False,
                    )
                nc.tensor.matmul(
                    ps[:, : HPC * DH],
                    lhsT=ones[0:1, :P], rhs=wv_b,
                    start=False, stop=True,
                )
                nc.scalar.copy(vv[:, lt, :], ps[:, : HPC * DH])
                v_emitted[0] += 1

            # pair-1 projection chains, emitted as half-chain pieces
            p1_chains = deque([(w, n) for w in ("q", "k") for n in range(NQ)])
            p1_open = [None, None, 0]  # (w, n), psum, m-pos

            def emit_p1_piece():
                if p1_open[0] is None:
                    if not p1_chains:
                        return False
                    p1_open[0] = p1_chains.popleft()
                    p1_open[1] = pp_ps.tile([P, 512], f32, name="p1p", tag="pp")
                    p1_open[2] = 0
                w_, n = p1_open[0]
                ps = p1_open[1]
                m0 = p1_open[2]
                th = wqk[w_ + "h"]
                tl = wqk[w_ + "l"]
                for m in range(m0, m0 + 4):
                    for vi, (lw, rx) in enumerate((
                        (th[:, 1, m, :], xh),
                        (tl[:, 1, m, :], xh),
                        (th[:, 1, m, :], xl),
                    )):
                        nc.tensor.matmul(
                            ps, lhsT=lw, rhs=rx[:, m, _ts(n, 512)],
                            start=(m == 0 and vi == 0), stop=False,
                        )
                p1_open[2] = m0 + 4
                if p1_open[2] >= MC:
                    proj_evac(1, w_, n, ps)
                    p1_open[0] = None
                return True

            def p1_left():
                return p1_chains or p1_open[0] is not None

            # ---------------- fused score/z/out stages ----------------
            ptgs = {}

            def emit_S_front(pr, g, s, h2):
                # one (head, q-tile): score matmuls + per-chunk max + exp
                if s == 0 and h2 == 0:
                    # layout [P, s, j, P]: transpose writes are contiguous
                    # (i+1)*256B runs per partition instead of 256B
                    ptgs[(pr, g)] = [
                        ptp.tile([P, G, LT, P], bf16, name=f"ptg{j}", tag=f"ptg{j}")
                        for j in range(2)
                    ]
                t = qkS[pr]
                i = g * G + s
                klen = (i + 1) * P
                nch = (klen + SCH - 1) // SCH
                lq = t["qS"][:, h2, _ts(i, P)]
                prow = prowp.tile([P, L], bf16)
                negmc = statp.tile([P, 4], f32, tag="negmc")
                sums = statp.tile([P, 4], f32, tag="sums")
                for c in range(nch):
                    cw = min(SCH, klen - c * SCH)
                    dlo = klen - P - c * SCH  # diag block offset in chunk
                    sp = s_ps.tile([P, SCH], f32, name="sp", tag="s")
                    for w0 in range(0, cw, 512):
                        ww = min(512, cw - w0)
                        has_diag = w0 <= dlo < w0 + ww
                        nc.tensor.matmul(
                            sp[:, w0 : w0 + ww], lhsT=lq,
                            rhs=t["kS"][:, h2, c * SCH + w0 : c * SCH + w0 + ww],
                            start=True, stop=False,
                        )
                        nc.tensor.matmul(
                            sp[:, w0 : w0 + ww], lhsT=lq,
                            rhs=t["kSw"][:, h2, c * SCH + w0 : c * SCH + w0 + ww],
                            start=False, stop=not has_diag,
                        )
                        if has_diag:
                            nc.tensor.matmul(
                                sp[:, dlo : dlo + P], lhsT=ident, rhs=mask,
                                start=False, stop=True,
                            )
                    nc.vector.reduce_max(
                        negmc[:, c : c + 1], sp[:, :cw], axis=AX.X, negate=True
                    )
                    nc.scalar.activation(
                        prow[:, c * SCH : c * SCH + cw],
                        sp[:, :cw],
                        AF.Exp,
                        bias=negmc[:, c : c + 1],
                        accum_out=sums[:, c : c + 1],
                    )
                return (pr, g, s, h2, prow, negmc, sums, nch, klen)

            def emit_S_back(ctx):
                # deferred per-unit tail: global rescale + 1/sum + transpose
                pr, g, s, h2, prow, negmc, sums, nch, klen = ctx
                i = g * G + s
                ptg = ptgs[(pr, g)]
                sinv = statp.tile([P, 1], f32, tag="sinv")
                if nch > 1:
                    negmg = statp.tile([P, 1], f32, tag="negmg")
                    nc.vector.tensor_reduce(
                        negmg, negmc[:, :nch], axis=AX.X, op=mybir.AluOpType.min
                    )
                    rsc = statp.tile([P, 4], f32, tag="rsc")
                    nc.scalar.activation(
                        rsc[:, :nch], negmc[:, :nch], AF.Exp,
                        bias=negmg, scale=-1.0,
                    )
                    ssc = statp.tile([P, 4], f32, tag="ssc")
                    nc.vector.tensor_mul(ssc[:, :nch], sums[:, :nch], rsc[:, :nch])
                    stot = statp.tile([P, 1], f32, tag="stot")
                    nc.vector.reduce_sum(stot, ssc[:, :nch], axis=AX.X)
                    nc.vector.reciprocal(sinv, stot)
                    wsc = statp.tile([P, 4], f32, tag="wsc")
                    nc.vector.tensor_scalar_mul(wsc[:, :nch], rsc[:, :nch], sinv)
                    for c in range(nch):
                        cw = min(SCH, klen - c * SCH)
                        nc.vector.tensor_scalar_mul(
                            prow[:, c * SCH : c * SCH + cw],
                            prow[:, c * SCH : c * SCH + cw],
                            wsc[:, c : c + 1],
                        )
                else:
                    nc.vector.reciprocal(sinv, sums[:, :1])
                    nc.vector.tensor_scalar_mul(
                        prow[:, :klen], prow[:, :klen], sinv
                    )
                nc.sync.dma_start_transpose(
                    ptg[h2][:, s, : i + 1, :], prow[:, :klen]
                )

            def emit_Z_h(pr, g, h2):
                ptg = ptgs[(pr, g)]
                hcol = (pr * 2 + h2) * DH
                zps = zo_ps.tile([DH, G * P], f32, name="zps", tag="zo")
                jmax = G * (g + 1)
                for j in range(jmax):
                    sc = max(0, j - G * g)
                    nc.tensor.matmul(
                        zps[:, sc * P :],
                        lhsT=vv[:, j, hcol : hcol + DH],
                        rhs=ptg[h2][:, sc:G, j, :],
                        start=(j == 0),
                        stop=(j == jmax - 1),
                    )
                nc.vector.tensor_scalar_mul(zst[g][_ts(h2, DH), pr, :], zps, 1.0)

            def emit_O_qtile(g, s):
                i = g * G + s
                osb = osbp.tile([P, DM], bf16)
                for mc2 in range(2):
                    ops = zo_ps.tile([P, 512], f32, name="ops", tag="zo")
                    for pr in range(NPAIR):
                        nc.tensor.matmul(
                            ops,
                            lhsT=zst[g][:, pr, _ts(s, P)],
                            rhs=wo_t[:, pr, _ts(mc2, 512)],
                            start=(pr == 0),
                            stop=(pr == 1),
                        )
                    if mc2 == 0:
                        nc.scalar.copy(osb[:, _ts(mc2, 512)], ops)
                    else:
                        nc.vector.tensor_scalar_mul(osb[:, _ts(mc2, 512)], ops, 1.0)
                nc.gpsimd.dma_start(out[_ts(i, P), :], osb)

            # Deferred z / out-proj work popped between score units so the
            # PE always has independent matmuls while softmax drains.
            zq = deque()
            oq = deque()
            epoch = [0]

            def z_poppable(drain):
                # z(pr,g) may only be emitted once its group's transposes are
                # queued (epoch gate) and its vv tiles are already emitted
                # (PE-queue FIFO would deadlock otherwise).
                if not zq:
                    return False
                ep, (pr_, g_, h2_) = zq[0]
                if not (drain or ep <= epoch[0] - 1):
                    return False
                return v_emitted[0] >= G * (g_ + 1)

            def pump(budget, stage, drain=False):
                while budget > 0:
                    chain_open = p1_open[0] is not None
                    need_v = min(G * (epoch[0] + 2), LT)
                    if stage == 1 and v_emitted[0] < need_v and not chain_open:
                        # v tiles needed by upcoming z chains; a new pp-pool
                        # alloc is only safe with no half-done p1 chain open
                        emit_v_tile(v_emitted[0])
                        budget -= 1
                    elif z_poppable(drain):
                        pr_, g_, h2_ = zq.popleft()[1]
                        emit_Z_h(pr_, g_, h2_)
                        if pr_ == 1 and h2_ == 1:
                            for s_ in range(G):
                                oq.append((g_, s_))
                        budget -= max(1, (G * (g_ + 1)) // 6)
                    elif stage == 1 and p1_left():
                        emit_p1_piece()
                        budget -= 2
                    elif stage == 1 and v_emitted[0] < LT and not chain_open:
                        emit_v_tile(v_emitted[0])
                        budget -= 1
                    elif stage == 2 and oq:
                        emit_O_qtile(*oq.popleft())
                        budget -= 1
                    else:
                        return

            pending = [None]

            def emit_S(pr, g, stage):
                # safety drain: a z chain two epochs stale must be emitted
                # before this block's ptg alloc (PE-queue FIFO would
                # deadlock on the ptg pool rotation otherwise)
                while zq and zq[0][0] <= epoch[0] - 2:
                    _, (pr_, g_, h2_) = zq[0]
                    while p1_open[0] is not None:
                        emit_p1_piece()
                    while v_emitted[0] < G * (g_ + 1):
                        emit_v_tile(v_emitted[0])
                    zq.popleft()
                    emit_Z_h(pr_, g_, h2_)
                    if pr_ == 1 and h2_ == 1:
                        for s_ in range(G):
                            oq.append((g_, s_))
                # software pipeline: unit u+1's matmuls+max+exp are emitted
                # before unit u's stat/rescale/transpose tail.
                for s_ in range(G):
                    for h2 in range(2):
                        ctx = emit_S_front(pr, g, s_, h2)
                        if pending[0] is not None:
                            emit_S_back(pending[0])
                        pending[0] = ctx
                        pump(3, stage)
                for h2 in range(2):
                    zq.append((epoch[0], (pr, g, h2)))
                epoch[0] += 1

            # stage 1: pair-0 groups 0..NG-3 with proj-1/v/z filler
            for g in range(NG - 2):
                emit_S(0, g, stage=1)

            # boundary: drain leftover proj-1/v filler (pair-1 units need it)
            if pending[0] is not None:
                emit_S_back(pending[0])
                pending[0] = None
            while p1_left():
                emit_p1_piece()
            while v_emitted[0] < LT:
                emit_v_tile(v_emitted[0])

            # stage 2: pair-1 groups interleaved with pair-0's two biggest
            # groups (their units are meaty PE filler while pair-1's early
            # softmax tails drain, and they keep out-tiles flowing)
            for pr_g in [(1, 0), (0, NG - 2), (1, 1), (0, NG - 1)] + [
                (1, g) for g in range(2, NG)
            ]:
                emit_S(pr_g[0], pr_g[1], stage=2)
            if pending[0] is not None:
                emit_S_back(pending[0])
                pending[0] = None
            while zq:
                pump(1, stage=2, drain=True)
                for _ in range(4):
                    if oq:
                        emit_O_qtile(*oq.popleft())
            while oq:
                emit_O_qtile(*oq.popleft())

    nc.finalize()
    return nc


def _split_bf16(a):
    import ml_dtypes

    hi = a.astype(ml_dtypes.bfloat16)
    lo = (a - hi.astype(np.float32)).astype(ml_dtypes.bfloat16)
    return hi, lo


def make_in_maps(normal_pre_resid, W_Q, W_K, W_V, W_O, b_Q, b_K, b_V, b_O):
    import ml_dtypes

    x = np.asarray(normal_pre_resid, np.float32)
    W_Q = np.asarray(W_Q, np.float32) * 0.125  # fold 1/sqrt(d_head)
    W_K = np.asarray(W_K, np.float32)
    W_V = np.asarray(W_V, np.float32)
    W_O = np.asarray(W_O, np.float32)
    b_Q = np.asarray(b_Q, np.float32) * 0.125
    b_K = np.asarray(b_K, np.float32)
    b_V = np.asarray(b_V, np.float32)

    mask = np.triu(np.full((P, P), NEG, np.float32), k=1).astype(ml_dtypes.bfloat16)
    ident = np.eye(P, dtype=np.float32).astype(ml_dtypes.bfloat16)
    in_maps = []
    for c in range(8):
        b, hg = divmod(c, 4)
        heads = [4 * hg + j for j in range(HPC)]
        xT = np.ascontiguousarray(x[b].T)  # [DM, L]
        xh, xl = _split_bf16(xT)

        def pack_qk(W, bias):
            prs = []
            for p_ in range(NPAIR):
                h0, h1 = heads[2 * p_], heads[2 * p_ + 1]
                wcat = np.concatenate([W[h0], W[h1]], axis=1)  # [DM, 128]
                bcat = np.concatenate([bias[h0], bias[h1]])[None, :]
                prs.append(np.concatenate([wcat, bcat], axis=0))  # [DM+1, 128]
            return _split_bf16(np.ascontiguousarray(np.stack(prs)))

        wqh, wql = pack_qk(W_Q, b_Q)
        wkh, wkl = pack_qk(W_K, b_K)
        wv_cat = np.concatenate([W_V[h] for h in heads], axis=1)
        bv_cat = np.concatenate([b_V[h] for h in heads])[None, :]
        wv_full = np.concatenate([wv_cat, bv_cat], axis=0).astype(ml_dtypes.bfloat16)
        wo_prs = np.ascontiguousarray(
            np.stack(
                [
                    np.concatenate(
                        [W_O[heads[2 * p_]], W_O[heads[2 * p_ + 1]]], axis=0
                    )
                    for p_ in range(NPAIR)
                ]
            )
        ).astype(ml_dtypes.bfloat16)  # [2, 128, DM]

        in_maps.append(
            {
                "xh": np.ascontiguousarray(xh),
                "xl": np.ascontiguousarray(xl),
                "wqh": wqh,
                "wql": wql,
                "wkh": wkh,
                "wkl": wkl,
                "wv": np.ascontiguousarray(wv_full),
                "wo": wo_prs,
                "mask": mask,
                "ident": ident,
            }
        )
    return in_maps


def run_device(in_maps, **kwargs):
    from concourse.bass_utils import run_bass_kernel_spmd

    if "nc" not in _CACHE:
        _CACHE["nc"] = build_bass()
    return run_bass_kernel_spmd(_CACHE["nc"], in_maps, core_ids=list(range(8)), **kwargs)


def kernel(normal_pre_resid, W_Q, W_K, W_V, W_O, b_Q, b_K, b_V, b_O, **extra):
    b_O = np.asarray(b_O, np.float32)
    in_maps = make_in_maps(
        normal_pre_resid, W_Q, W_K, W_V, W_O, b_Q, b_K, b_V, b_O
    )
    res = run_device(in_maps)
    outs = [r["out"] for r in res.results]
    full = np.zeros((B, L, DM), np.float32)
    for c in range(8):
        full[c // 4] += outs[c].astype(np.float32)
    full += b_O[None, None, :]
    return full
